# revision 19
# baseline (speedup 1.0000x reference)
"""CenterLoss on 8 TRN2 NeuronCores.

loss = mean_i clip(||x_i - centers[labels_i]||^2, 1e-12, 1e12)

Strategy (data-parallel, per sharding hint):
 - shard x/labels along batch: 4096 rows per core; centers (200MB) replicated.
 - per core: load the x shard into SBUF once (8MB), gather the 4096 needed
   center rows with indirect DMA (128 rows / 2KB each per instruction),
   diff on DVE, square+row-sum fused on the scalar engine (ACT accum_out),
   final [128,1] per-partition partial sums DMA'd out.
 - host: sum the 8x128 partials, divide by B.
"""

import numpy as np

import concourse.bacc as bacc
import concourse.bass as bass
import concourse.mybir as mybir
import concourse.tile as tile
from concourse.bass_utils import run_bass_kernel_spmd

B = 32768
F = 512
C = 100000
NCORES = 8
BPC = B // NCORES  # 4096 rows per core
P = 128
T = BPC // P  # 32 column-tiles per core

f32 = mybir.dt.float32
i32 = mybir.dt.int32


def build(bpc: int = BPC, feat: int = F, ncls: int = C) -> bass.Bass:
    t_tiles = bpc // P
    nc = bacc.Bacc(None, target_bir_lowering=False, num_swdge_queues=2)
    x = nc.declare_dram_parameter("x", [bpc, feat], f32, isOutput=False)
    labels = nc.declare_dram_parameter("labels", [bpc], i32, isOutput=False)
    centers = nc.declare_dram_parameter("centers", [ncls, feat], f32, isOutput=False)
    out = nc.declare_dram_parameter("out", [P, 1], f32, isOutput=True)

    chunk = min(4, t_tiles)  # tiles per x-load chunk (1MB per dma_start)
    n_chunks = (t_tiles + chunk - 1) // chunk
    with tile.TileContext(nc) as tc:
        with (
            tc.tile_pool(name="big", bufs=1) as big,
            tc.tile_pool(name="xc", bufs=3) as xc,
            tc.tile_pool(name="cg", bufs=8) as cg,
            tc.tile_pool(name="work", bufs=8) as work,
        ):
            # x viewed as [P, t_tiles, feat] with row index p*t_tiles + t:
            # contiguous per partition; loaded in 1MB chunks so each compute
            # tile waits on a single DMA semaphore.
            xv = x[:].rearrange("(p t) f -> p t f", p=P)
            lab = big.tile([P, t_tiles], i32)
            acc = big.tile([P, t_tiles], f32)
            nc.sync.dma_start(
                out=lab[:], in_=labels[:].rearrange("(p t) -> p t", p=P)
            )
            for ci in range(n_chunks):
                t0 = ci * chunk
                t1 = min(t0 + chunk, t_tiles)
                nt = t1 - t0
                x_chunk = xc.tile([P, chunk * feat], f32, tag="x")
                nc.sync.dma_start(
                    out=x_chunk[:, : nt * feat],
                    in_=xv[:, t0:t1, :].rearrange("p t f -> p (t f)"),
                )
                # NOTE: the HW indirect-DMA ucode consumes ONE offset per dest
                # partition row and streams the rest contiguously (CoreSim's
                # flat multi-offset model does NOT match HW) — so each gather
                # must be [P, feat] with a [P, 1] offset column.
                for j in range(nt):
                    t = t0 + j
                    c_tile = cg.tile([P, feat], f32, tag="c")
                    diff = work.tile([P, feat], f32, tag="d")
                    sq = work.tile([P, feat], f32, tag="s")
                    gi = nc.gpsimd.indirect_dma_start(
                        out=c_tile[:],
                        out_offset=None,
                        in_=centers[:],
                        in_offset=bass.IndirectOffsetOnAxis(
                            ap=lab[:, t : t + 1], axis=0
                        ),
                    )
                    # alternate the two SWDGE FIFO contexts so descriptor
                    # service on one doesn't backpressure the next issue
                    if t % 2:
                        gi.ins.queue = "qPoolDynamic1"
                    nc.vector.tensor_tensor(
                        out=diff[:],
                        in0=x_chunk[:, j * feat : (j + 1) * feat],
                        in1=c_tile[:],
                        op=mybir.AluOpType.subtract,
                    )
                    nc.scalar.activation(
                        out=sq[:],
                        in_=diff[:],
                        func=mybir.ActivationFunctionType.Square,
                        accum_out=acc[:, t : t + 1],
                    )
            # clamp per-row dist like the reference, then sum the row dists
            accv = big.tile([P, 1], f32)
            nc.vector.tensor_scalar(
                out=acc[:],
                in0=acc[:],
                scalar1=1e-12,
                scalar2=1e12,
                op0=mybir.AluOpType.max,
                op1=mybir.AluOpType.min,
            )
            nc.vector.tensor_reduce(
                out=accv[:],
                in_=acc[:],
                axis=mybir.AxisListType.X,
                op=mybir.AluOpType.add,
            )
            nc.sync.dma_start(out=out[:], in_=accv[:])
    nc.finalize()
    return nc


def kernel(x, labels, centers):
    nc = build()
    xs = np.ascontiguousarray(np.asarray(x, dtype=np.float32))
    labs = np.ascontiguousarray(np.asarray(labels).astype(np.int32))
    cens = np.ascontiguousarray(np.asarray(centers, dtype=np.float32))
    in_maps = []
    for k in range(NCORES):
        sl = slice(k * BPC, (k + 1) * BPC)
        in_maps.append(
            {
                "x": np.ascontiguousarray(xs[sl]),
                "labels": np.ascontiguousarray(labs[sl]),
                "centers": cens,
            }
        )
    res = run_bass_kernel_spmd(nc, in_maps, core_ids=list(range(NCORES)))
    total = sum(float(np.sum(r["out"], dtype=np.float64)) for r in res.results)
    return np.asarray(total / B, dtype=np.float32)
